# revision 43
# baseline (speedup 1.0000x reference)
"""MoE gate routing kernel for Trainium2 (8 NeuronCores, Bass/Tile).

Computes, for hidden_states [4, 4096, 7168] (f32), gate kernel [7168, 256],
e_score_correction_bias [256]:
    logits = x @ W ; scores = sigmoid(logits) + bias
    grouped top-2-sum -> top-4 groups of 8 -> mask -> top-8 experts
    weights = 2.5 * topk_vals / sum(topk_vals)
Returns (topk_idx int32 [16384, 8], topk_weight f32 [16384, 8]).

Sharding: tokens split evenly across 8 cores (2048 tokens/core); W + bias
replicated. No cross-core communication.

GEMM precision strategy: the PE runs float32r (TF32-like, ~11-bit mantissa)
matmuls at 1 cycle/row vs 4 for fp32.  A single f32r pass flips ~0.5% of the
routing decisions (group top-4 ties), which blows the weight error to ~2.5e-2.
Instead we do an exact-grade 3-pass split:
    x = x_hi + x_lo,  W = w_hi + w_lo   (hi = f32r-rounded, lo = residual)
    logits = x_hi@w_hi + x_hi@w_lo + x_lo@w_hi   (x_lo@w_lo ~ 2^-22, dropped)
which reproduces fp32 routing bit-for-bit on this dataset (rel err 3.5e-7)
at 3 cycles/row.  x is transposed once on the PE in exact fp32; the hi part
is formed by the PSUM->SBUF copy on the scalar engine rounding to f32r, and
the lo part by a vector-engine subtract, also rounding to f32r.
"""
import sys

sys.path.insert(0, "/opt/trn_rl_repo")

import numpy as np

import concourse.bass as bass  # noqa: F401  (engine types referenced via nc)
import concourse.mybir as mybir
import concourse.tile as tile
from concourse import bacc
from concourse.bass_utils import run_bass_kernel_spmd

# Problem constants (hardcoded per contract)
H = 7168
E = 256
N_CORES = 8
T_FULL = 4 * 4096           # 16384 tokens
T_C = T_FULL // N_CORES     # 2048 tokens per core
P = 128
KT = H // P                 # 56 contraction tiles
TT = T_C // P               # 16 token tiles per core
GROUP = 4                   # k-tiles per transpose psum tile
NG = KT // GROUP            # 14
PIPE = 5                    # transpose groups in flight ahead of matmuls
XCH = 4 * P                 # x DMA chunk width (4 k-tiles); 14 chunks for tile 0
WCH = 2                     # k-tiles per W prep chunk
N_GROUP = 8
TOPK_GROUP = 4
TOP_K = 8
EPG = E // N_GROUP          # 32 experts per group
SCALE = 2.5

f32 = mybir.dt.float32
f32r = mybir.dt.float32r    # TF32-like on PE; 1 cyc/row vs 4 for fp32
u32 = mybir.dt.uint32

_CACHED_NC = None


def _build_nc():
    nc = bacc.Bacc("TRN2", target_bir_lowering=False, debug=False)
    # x and w stay genuinely f32-typed on chip until the hi/lo split: the
    # scalar engine's f32->f32r copy is what performs the TF32-style rounding
    # (f32r->f32r copies and bitcast views pass bits through unrounded).
    x = nc.dram_tensor("x", [T_C, H], f32, kind="ExternalInput")
    w = nc.dram_tensor("w", [H, E], f32, kind="ExternalInput")
    b = nc.dram_tensor("b", [E], f32, kind="ExternalInput")
    ident_dram = nc.dram_tensor("ident", [P, P], f32, kind="ExternalInput")
    idx_out = nc.dram_tensor("idx_out", [T_C, TOP_K], u32, kind="ExternalOutput")
    wt_out = nc.dram_tensor("wt_out", [T_C, TOP_K], f32, kind="ExternalOutput")

    w_kpe = w.ap().rearrange("(ko p) e -> p ko e", p=P)  # [128, 56, 256]
    # p-major token mapping: tile t holds tokens {p*TT + t}. Makes the final
    # output DMA per-partition lines contiguous (TT*8 elems = 512B).
    x_tp = x.ap().rearrange("(p t) h -> p t h", t=TT)    # [128, 16, 7168]
    idx_tp = idx_out.ap().rearrange("(p t) k -> p t k", t=TT)
    wt_tp = wt_out.ap().rearrange("(p t) k -> p t k", t=TT)

    with tile.TileContext(nc) as tc:
        with (
            tc.tile_pool(name="const", bufs=1) as cpool,
            tc.tile_pool(name="xp", bufs=5) as x_pool,
            tc.tile_pool(name="xth", bufs=4) as xth_pool,
            tc.tile_pool(name="xtl", bufs=4) as xtl_pool,
            tc.tile_pool(name="whc", bufs=2) as whc_pool,
            tc.tile_pool(name="sc", bufs=1) as sc_pool,
            tc.tile_pool(name="tk", bufs=2) as tk_pool,
            tc.tile_pool(name="outp", bufs=1) as out_pool,
            tc.tile_pool(name="ps_l", bufs=2, space="PSUM") as ps_logits,
            tc.tile_pool(name="ps_t", bufs=6, space="PSUM") as ps_tr,
        ):
            # fp32 identity (exact transposes), DMA'd from DRAM.
            ident = cpool.tile([P, P], f32)
            nc.sync.dma_start(ident, ident_dram.ap())

            # DMA order: x tile-0 chunks first (PE starts transposing as soon
            # as the first lands), W streamed behind it in consumption order
            # (chunk g feeds matmul group g of tile 0; later W chunks are
            # emitted from inside the pipeline loop so they don't delay the
            # tile-1 x DMAs on the in-order sync queue).  w_sb keeps the raw
            # fp32 bits and acts as the hi part (PE read rounds to f32r);
            # w_lo = W - round_f32r(W) is formed chunk-wise on-chip.
            HH = H // 2
            x_halves = {}
            xh00 = x_pool.tile([P, HH], f32, tag="x_h")
            xh01 = x_pool.tile([P, HH], f32, tag="x_h")
            x_halves[(0, 0)] = xh00
            x_halves[(0, 1)] = xh01
            w_hi = cpool.tile([P, KT, E], f32r)
            w_lo = cpool.tile([P, KT, E], f32r)

            # tile-0 chunk plan (k-tiles): finer first chunks so the first
            # transposes start as early as possible
            X0_CHUNKS = [(0, 2), (2, 2), (4, 3), (7, 7), (14, 7), (21, 7),
                         (28, 7), (35, 7), (42, 7), (49, 7)]

            def x0c(c):
                kt0, nkt = X0_CHUNKS[c]
                h = kt0 >= KT // 2
                sl = slice(kt0 * P - h * HH, (kt0 + nkt) * P - h * HH)
                gsl = slice(kt0 * P, (kt0 + nkt) * P)
                nc.sync.dma_start(x_halves[(0, h)][:, sl], x_tp[:, 0, gsl])

            w_chunks = {}

            def wchunk_dma(wb):
                wc = whc_pool.tile([P, WCH, E], f32, tag="whc")
                nc.sync.dma_start(wc, w_kpe[:, wb : wb + WCH])
                w_chunks[wb] = wc

            def wchunk_split(wb):
                wc = w_chunks.pop(wb)
                # hi = f32r-rounded W (scalar engine rounds fp32->f32r on
                # the copy-out)
                nc.scalar.copy(out=w_hi[:, wb : wb + WCH], in_=wc)
                # lo = W - hi, rounded to f32r on write
                nc.vector.tensor_sub(
                    w_lo[:, wb : wb + WCH],
                    wc,
                    w_hi[:, wb : wb + WCH].bitcast(f32),
                )

            for c in (0, 1, 2, 3):
                x0c(c)
            wchunk_dma(0)
            x0c(4); x0c(5)
            wchunk_dma(WCH)
            wchunk_split(0)
            x0c(6); x0c(7)
            wchunk_split(WCH)
            for c in range(8, len(X0_CHUNKS)):
                x0c(c)

            bias_sb = cpool.tile([P, E], f32)
            nc.sync.dma_start(bias_sb, b.ap().unsqueeze(0).partition_broadcast(P))

            idx_acc = out_pool.tile([P, TT, TOP_K], u32)
            wt_acc = out_pool.tile([P, TT, TOP_K], f32)

            def topk_chain(t, logits):
                """Fused sigmoid+bias+grouped-top-k for one 128-token tile."""
                scores = sc_pool.tile([P, E], f32, tag="scores")
                nc.scalar.activation(
                    out=scores, in_=logits, func=mybir.ActivationFunctionType.Sigmoid
                )
                nc.vector.tensor_add(scores, scores, bias_sb)

                gmax8 = tk_pool.tile([P, N_GROUP, 8], f32, tag="gmax8")
                for g in range(N_GROUP):
                    nc.vector.max(out=gmax8[:, g], in_=scores[:, g * EPG : (g + 1) * EPG])
                gsum = tk_pool.tile([P, N_GROUP], f32, tag="gsum")
                nc.vector.tensor_add(gsum, gmax8[:, :, 0], gmax8[:, :, 1])
                gs8 = tk_pool.tile([P, 8], f32, tag="gs8")
                nc.vector.max(out=gs8, in_=gsum)
                gmask = tk_pool.tile([P, N_GROUP], f32, tag="gmask")
                nc.vector.tensor_scalar(
                    out=gmask, in0=gsum,
                    scalar1=gs8[:, TOPK_GROUP - 1 : TOPK_GROUP], scalar2=None,
                    op0=mybir.AluOpType.is_ge,
                )
                masked = sc_pool.tile([P, E], f32, tag="masked")
                nc.vector.tensor_mul(
                    masked.rearrange("p (g j) -> p g j", g=N_GROUP),
                    scores.rearrange("p (g j) -> p g j", g=N_GROUP),
                    gmask.unsqueeze(2).to_broadcast([P, N_GROUP, EPG]),
                )
                vals8 = tk_pool.tile([P, 8], f32, tag="vals8")
                nc.vector.max(out=vals8, in_=masked)
                nc.vector.max_index(out=idx_acc[:, t], in_max=vals8, in_values=masked)
                if t == TT - 1:
                    # last tile: idx leaves as soon as it's ready, overlapping
                    # the weight-normalization tail
                    nc.sync.dma_start(idx_tp[:, TT - 1 :], idx_acc[:, TT - 1 :])
                denom = tk_pool.tile([P, 1], f32, tag="denom")
                nc.vector.reduce_sum(out=denom, in_=vals8, axis=mybir.AxisListType.X)
                inv = tk_pool.tile([P, 1], f32, tag="inv")
                nc.vector.reciprocal(inv, denom)
                nc.vector.tensor_scalar(
                    out=wt_acc[:, t], in0=vals8,
                    scalar1=inv[:, 0:1], scalar2=SCALE,
                    op0=mybir.AluOpType.mult, op1=mybir.AluOpType.mult,
                )
                if t == TT - 2:
                    # bulk of the output leaves while the last tile computes
                    nc.sync.dma_start(idx_tp[:, : TT - 1], idx_acc[:, : TT - 1])
                    nc.sync.dma_start(wt_tp[:, : TT - 1], wt_acc[:, : TT - 1])
                elif t == TT - 1:
                    nc.sync.dma_start(wt_tp[:, TT - 1 :], wt_acc[:, TT - 1 :])

            # Flat software pipeline over all (tile, group) pairs: transposes
            # + hi/lo extraction run PIPE groups ahead of the matmuls that
            # consume them, with no barrier at t-tile boundaries.
            TOTAL = TT * NG
            xt_hi_flat = [None] * TOTAL
            xt_lo_flat = [None] * TOTAL
            logits_of = {}

            def emit_mms(Gf):
                t, g = divmod(Gf, NG)
                xh = xt_hi_flat[Gf]
                xl = xt_lo_flat[Gf]
                logits = logits_of[t]
                for i in range(GROUP):
                    kt = g * GROUP + i
                    sl = slice(i * P, (i + 1) * P)
                    nc.tensor.matmul(
                        logits, xh[:, sl], w_hi[:, kt, :],
                        start=(kt == 0), stop=False,
                    )
                    nc.tensor.matmul(
                        logits, xh[:, sl], w_lo[:, kt, :],
                        start=False, stop=False,
                    )
                    nc.tensor.matmul(
                        logits, xl[:, sl], w_hi[:, kt, :],
                        start=False, stop=(kt == KT - 1),
                    )
                if g == NG - 1:
                    topk_chain(t, logits)

            for Gf in range(TOTAL + PIPE):
                if Gf < TOTAL:
                    t, g = divmod(Gf, NG)
                    if g == 0:
                        logits = ps_logits.tile([P, E], f32, tag="logits")
                        logits_of[t] = logits
                    if g == 7 and t < TT - 1:
                        # prefetch next tile's x halves mid-tile
                        for h in (0, 1):
                            xt_h = x_pool.tile([P, HH], f32, tag="x_h")
                            nc.sync.dma_start(
                                xt_h, x_tp[:, t + 1, h * HH : (h + 1) * HH]
                            )
                            x_halves[(t + 1, h)] = xt_h
                    half = x_halves[(t, g >= 7)]
                    pst = ps_tr.tile([P, GROUP * P], f32, tag="pst")
                    for i in range(GROUP):
                        kt = g * GROUP + i
                        off = kt * P - (HH if g >= 7 else 0)
                        nc.tensor.transpose(
                            pst[:, i * P : (i + 1) * P],
                            half[:, off : off + P],
                            ident,
                        )
                    # hi: scalar-engine copy rounds fp32 -> f32r
                    xh = xth_pool.tile([P, GROUP * P], f32r, tag="xth")
                    nc.scalar.copy(out=xh, in_=pst)
                    # lo: exact fp32 residual, rounded to f32r on write
                    xl = xtl_pool.tile([P, GROUP * P], f32r, tag="xtl")
                    nc.vector.tensor_sub(xl, pst, xh.bitcast(f32))
                    xt_hi_flat[Gf] = xh
                    xt_lo_flat[Gf] = xl
                    # stream the remaining W chunks behind the pipeline work
                    # (emitted last so the in-order Act/DVE queues service
                    # the xh/xl ops first); two chunks per group iteration
                    for ci in (2 * Gf + 2, 2 * Gf + 3):
                        if 2 <= ci < KT // WCH:
                            wchunk_dma(ci * WCH)
                            wchunk_split(ci * WCH)
                if Gf >= PIPE:
                    emit_mms(Gf - PIPE)

    nc.compile()
    return nc


def get_nc():
    global _CACHED_NC
    if _CACHED_NC is None:
        _CACHED_NC = _build_nc()
    return _CACHED_NC


def run(hidden_states, kernel_w, bias, trace=False, trace_cores=None):
    """Internal entry that also exposes trace results for benchmarking."""
    x_full = np.ascontiguousarray(
        np.asarray(hidden_states, dtype=np.float32).reshape(T_FULL, H)
    )
    w_np = np.ascontiguousarray(np.asarray(kernel_w, dtype=np.float32))
    b_np = np.ascontiguousarray(np.asarray(bias, dtype=np.float32))

    nc = get_nc()
    ident_np = np.eye(P, dtype=np.float32)
    in_maps = [
        {"x": x_full[c * T_C : (c + 1) * T_C], "w": w_np, "b": b_np, "ident": ident_np}
        for c in range(N_CORES)
    ]
    kw = {}
    if trace:
        kw = dict(trace=True, trace_cores=trace_cores or [0])
    last_err = None
    for attempt in range(3):
        try:
            res = run_bass_kernel_spmd(nc, in_maps, core_ids=list(range(N_CORES)), **kw)
            break
        except Exception as e:  # transient NRT/axon device hiccups
            last_err = e
            if attempt == 2:
                raise
            import time as _time

            _time.sleep(15)
    else:
        raise last_err

    idx = np.concatenate([r["idx_out"] for r in res.results], axis=0).astype(np.int32)
    wt = np.concatenate([r["wt_out"] for r in res.results], axis=0)
    return (idx, wt), res


def kernel(hidden_states, kernel, e_score_correction_bias):
    (idx, wt), _ = run(hidden_states, kernel, e_score_correction_bias)
    return idx, wt
